# revision 9
# baseline (speedup 1.0000x reference)
"""Trainium2 Bass kernel: causal multi-head attention with softmax over the
QUERY axis (faithful to the reference's softmax(dim=-2) quirk).

Problem shapes: B=2, T=2048, E=1024, H=16, D=64.

Sharding: 8 cores = 2 batches x 4 head-groups (4 heads per core, zero
communication).  Host pre-transposes X to (E, T) per batch and repacks it
(and the weights) into the exact SBUF partition-major layout so every input
DMA is a fully contiguous row copy.

v4 (from v2's 172us / v3's 161us):
  - score matmuls use K=64/M=128 per head (lhsT = kt[64h:64h+64, s0:s0+128])
    -> 2 matmuls per 512-col chunk instead of 4 64x64 quadrants: halves the
    PE column-cycles spent on scores (58us -> 29us).
  - projections stay fp16 (v3 tried fp8 DoubleRow: on real HW DR streams the
    doubled rhs at 1 col/cycle, i.e. 2x SLOWER than fp16 for the same math,
    and the measured cost-model 0.5 cyc/row did not materialize).
  - pair-1 Q/K feeder work is split into 8 half-jobs (q-only / k-only 512-col
    chunks) spread over units 1..8 so early-unit PE load stays below the ACT
    exp pace and the exp stream never stalls.

v2 schedule retained (single continuous stream, ACT-paced):
  - phase B projects ONLY pair 0's Q_T/K_T so the exp stream starts early;
    V projections and pair-1's Q/K are PE "feeder" jobs between early units.
  - causal mask applied on PE (identity @ mask matmul into the diagonal
    score block).
  - exp on ACT with fused accum_out => denominators come free.
  - O_T per pair accumulated in a rotating 2-bank PSUM half with the
    deferred high-t AV trick.
"""

import numpy as np
from contextlib import ExitStack

B, T, E, H, D = 2, 2048, 1024, 16, 64
NCORES = 8
PAIRS = 2          # head pairs per core (4 heads)
EC = E // 128      # 8 contraction chunks
TB = T // 128      # 16 s-blocks
TC = T // 512      # 4 output column chunks
SCALE = float(D) ** -0.5

_CACHE = {}


def _st_chunks(h0, h1):
    """Split [h0, h1) into <=512 pieces aligned to the psum tile's own 512
    grid (tile column 0 is at absolute t=h0)."""
    out = []
    c0 = h0
    while c0 < h1:
        w = min(512, h1 - c0)
        out.append((c0, w))
        c0 += w
    return out


def _av_chunks(s0):
    """Split [s0, 2048) on the absolute 512 grid (psum_o bank alignment)."""
    out = []
    for j in range(s0 // 512, TC):
        c0 = max(s0, 512 * j)
        out.append((j, c0, 512 * (j + 1) - c0))
    return out


def _emit(tc, io):
    """Emit the kernel program into TileContext tc.  io: dict name -> AP."""
    import concourse.bass as bass
    import concourse.mybir as mybir

    nc = tc.nc
    fp32 = mybir.dt.float32
    fp16 = mybir.dt.float16
    AF = mybir.ActivationFunctionType
    ALU = mybir.AluOpType

    x_t, wq, wk, wv = io["x_t"], io["wq"], io["wk"], io["wv"]
    bq, bk, bv, out = io["bq"], io["bk"], io["bv"], io["out"]

    with ExitStack() as ctx:
        const = ctx.enter_context(tc.tile_pool(name="const", bufs=1))
        big = ctx.enter_context(tc.tile_pool(name="big", bufs=1))
        epool = ctx.enter_context(tc.tile_pool(name="epool", bufs=11))
        opool = ctx.enter_context(tc.tile_pool(name="opool", bufs=2))
        small = ctx.enter_context(tc.tile_pool(name="small", bufs=24))
        vpool = ctx.enter_context(tc.tile_pool(name="vpool", bufs=18))
        # "ps" slots are (128,1024) = 2 banks each, bufs=3 -> 6 banks.
        pp = ctx.enter_context(tc.tile_pool(name="pp", bufs=3, space="PSUM"))
        # O_T half accumulator: (128,1024) = 2 banks, single slot.
        po = ctx.enter_context(tc.tile_pool(name="po", bufs=1, space="PSUM"))

        # ---------- constants + inputs ----------
        # X_T and weights arrive as fp16, packed host-side as (128, EC*n)
        # partition-major rows so every DMA is one contiguous copy.
        xt_sb = big.tile([128, EC, T], fp16, tag="xt")
        xt_view = x_t.rearrange("p (c t) -> p c t", c=EC)

        wq_sb = const.tile([128, EC, 256], fp16, tag="wq")
        wk_sb = const.tile([128, EC, 256], fp16, tag="wk")
        wv_sb = const.tile([128, EC, 256], fp16, tag="wv")
        # wq/wk arrive per e-chunk (interleaved, sync queue) and X chunk-by-
        # chunk (split gpsimd/sync queues), so the c-outer projection loop
        # starts once just chunk 0 of each has landed; wv + biases are tiny
        # and go on the (idle-early) scalar queue immediately.
        wq_view = wq.rearrange("p (c n) -> p c n", c=EC)
        wk_view = wk.rearrange("p (c n) -> p c n", c=EC)

        bq_sb = const.tile([128, PAIRS], fp32, tag="bq")
        bk_sb = const.tile([128, PAIRS], fp32, tag="bk")
        for p in range(PAIRS):
            nc.scalar.dma_start(out=bq_sb[:, p:p + 1], in_=bq[p, :, None])
            nc.scalar.dma_start(out=bk_sb[:, p:p + 1], in_=bk[p, :, None])
        bv_sb = const.tile([128, 256], fp32, tag="bv")
        bv_bcast = bass.AP(tensor=bv.tensor, offset=bv.offset,
                           ap=[[0, 128]] + list(bv.ap))
        nc.scalar.dma_start(out=bv_sb, in_=bv_bcast)
        nc.scalar.dma_start(out=wv_sb,
                            in_=wv.rearrange("p (c n) -> p c n", c=EC))

        nc.gpsimd.dma_start(out=wq_sb, in_=wq_view)
        nc.sync.dma_start(out=wk_sb, in_=wk_view)
        # x pieces round-robin over the three DMA-capable queues so chunk
        # arrival matches the c-outer projection order; total per queue is
        # ~1.5MB -> inputs land ~10us earlier than the 2-queue split
        xq = [nc.gpsimd, nc.sync, nc.scalar]
        for c in range(EC):
            for hh in range(2):
                q = xq[(2 * c + hh) % 3]
                q.dma_start(out=xt_sb[:, c, 1024 * hh:1024 * (hh + 1)],
                            in_=xt_view[:, c, 1024 * hh:1024 * (hh + 1)])

        # PE warm-up: dummy matmuls during the input-DMA wait so the HAM
        # clock gate reaches 2.4 GHz before real work starts
        warm_sb = const.tile([128, 512], fp16, tag="warm")
        nc.vector.memset(warm_sb, 0.0)
        wps = po.tile([128, 1024], fp32, tag="po", name="ps_warm")
        for _ in range(8):
            nc.tensor.matmul(wps[:, 0:512], lhsT=warm_sb[:, 0:128],
                             rhs=warm_sb, start=True, stop=True)

        # mask[s, t] = 0 if t >= s else MNEG, applied on PE as identity@mask
        # (a DVE mask-add head-of-line blocks the ACT queue behind feeder
        # bias-adds; the PE path keeps the exp stream self-contained)
        MNEG = -60000.0
        mask_sb = const.tile([128, 128], fp16, tag="mask")
        nc.vector.memset(mask_sb, 0.0)
        nc.gpsimd.affine_select(
            out=mask_sb, in_=mask_sb,
            pattern=[[1, 128]], channel_multiplier=-1, base=0,
            compare_op=ALU.is_ge, fill=MNEG,
        )
        ident_sb = const.tile([128, 128], fp16, tag="ident")
        nc.vector.memset(ident_sb, 0.0)
        nc.gpsimd.affine_select(
            out=ident_sb, in_=ident_sb,
            pattern=[[1, 128]], channel_multiplier=-1, base=0,
            compare_op=ALU.not_equal, fill=1.0,
        )

        # ---------- phase B: pair-0 Q/K projections ----------
        qt_sb = [big.tile([128, T], fp16, tag=f"qt{p}", name=f"qt{p}") for p in range(PAIRS)]
        kt_sb = [big.tile([128, T], fp16, tag=f"kt{p}", name=f"kt{p}") for p in range(PAIRS)]
        v_sb = big.tile([128, TB, 256], fp16, tag="v")

        # c-outermost over all 8 PSUM banks: the projection finishes ~2us
        # after the last X chunk lands instead of re-walking chunks twice.
        scq = [pp.tile([128, 1024], fp32, tag="ps", name="scq0"),
               pp.tile([128, 1024], fp32, tag="ps", name="scq1")]
        sck = [pp.tile([128, 1024], fp32, tag="ps", name="sck0"),
               po.tile([128, 1024], fp32, tag="po", name="sck1")]
        groups = ((scq, qt_sb[0], bq_sb, wq_sb), (sck, kt_sb[0], bk_sb, wk_sb))
        for c in range(EC):
            last = c == EC - 1
            for jh in range(2):
                for scs, dst, b_sb, w_sb in groups:
                    for j2 in range(2):
                        j = 2 * jh + j2
                        nc.tensor.matmul(
                            scs[j // 2][:, 512 * (j % 2):512 * (j % 2 + 1)],
                            lhsT=w_sb[:, c, 0:128],
                            rhs=xt_sb[:, c, 512 * j:512 * (j + 1)],
                            start=(c == 0), stop=last,
                        )
                        if last:
                            # bias-add j overlaps the next matmul's stream
                            nc.vector.tensor_scalar_add(
                                out=dst[:, 512 * j:512 * (j + 1)],
                                in0=scs[j // 2][:, 512 * (j % 2):
                                                512 * (j % 2 + 1)],
                                scalar1=b_sb[:, 0:1],
                            )

        # ---------- feeder jobs (run inside the attention stream) ----------
        vstate = {"next": 0}

        def emit_v_blocks(tbs):
            for tb in tbs:
                ps = pp.tile([128, 1024], fp32, tag="ps", name="ps_v")
                for c in range(EC):
                    nc.tensor.matmul(
                        ps[:, 0:256],
                        lhsT=xt_sb[:, c, 128 * tb:128 * (tb + 1)],
                        rhs=wv_sb[:, c, :],
                        start=(c == 0), stop=(c == EC - 1),
                    )
                nc.vector.tensor_tensor(out=v_sb[:, tb, :], in0=ps[:, 0:256],
                                        in1=bv_sb, op=ALU.add)

        def emit_v_upto(tb_needed):
            while vstate["next"] <= min(tb_needed, TB - 1):
                emit_v_blocks([vstate["next"]])
                vstate["next"] += 1

        def emit_qk1_half(job):
            """Project HALF of pair 1's Q or K for one 512-col t-chunk.
            job = (which in {q,k}, jc).  Half-jobs keep per-unit PE feeder
            load (~1.7us) below the ACT exp pace so exp never stalls."""
            which, jc = job
            w_sb, dst, b_sb = ((wq_sb, qt_sb[1], bq_sb) if which == "q"
                               else (wk_sb, kt_sb[1], bk_sb))
            sc = pp.tile([128, 1024], fp32, tag="ps", name=f"qk1_{which}{jc}")
            for c in range(EC):
                nc.tensor.matmul(
                    sc[:, 0:512], lhsT=w_sb[:, c, 128:256],
                    rhs=xt_sb[:, c, 512 * jc:512 * (jc + 1)],
                    start=(c == 0), stop=(c == EC - 1),
                )
            nc.vector.tensor_scalar_add(
                out=dst[:, 512 * jc:512 * (jc + 1)],
                in0=sc[:, 0:512], scalar1=b_sb[:, 1:2])

        # ---------- attention: one merged stream over 32 (pair, block) ----
        P = {}
        for p in range(PAIRS):
            P[p] = {"po_half": {}, "saved": {}, "o_sb": None,
                    "defer": None, "navs": 0, "f2": False, "f3": False}
        from collections import deque
        for p in range(PAIRS):
            P[p]["defer"] = deque()
            P[p]["o_sb"] = opool.tile([128, T], fp16, tag=f"o{p}",
                                      name=f"o_sb{p}")

        def open_half(p, ph):
            t = po.tile([128, 1024], fp32, tag="po", name=f"po_{p}_{ph}")
            P[p]["po_half"][ph] = t

        def emit_st_exp(p, i):
            s0 = 128 * i
            halves = [(h0, min(T, h0 + 1024))
                      for h0 in (s0, s0 + 1024) if h0 < T]
            e_t = [epool.tile([128, T], fp16, tag=f"e{h}", name=f"e{h}")
                   for h in range(2)]
            den = small.tile([128, 2, 2], fp32, tag="den", name="den")
            for hf, (h0, h1) in enumerate(halves):
                pss = [pp.tile([128, 1024], fp32, tag="ps", name="ps_st")
                       for _ in range(2)]
                # one K=64/M=128 matmul per head; the two heads sit in
                # opposite halves of the PE array (tile_position rows).
                # h-outer / chunk-inner keeps lhsT constant across chunks
                # so consecutive weight loads hit the already-loaded tile.
                for h in range(2):
                    for c0, cw in _st_chunks(h0, h1):
                        diag = hf == 0 and c0 == h0
                        nc.tensor.matmul(
                            pss[h][:, c0 - h0:c0 - h0 + cw],
                            lhsT=kt_sb[p][64 * h:64 * (h + 1), s0:s0 + 128],
                            rhs=qt_sb[p][64 * h:64 * (h + 1), c0:c0 + cw],
                            start=True, stop=True,
                            tile_position=(64 * h, 0),
                            skip_group_check=True,
                        )
                    if hf == 0:
                        # causal mask add on PE: += identity.T @ mask
                        nc.tensor.matmul(
                            pss[h][:, 0:128],
                            lhsT=ident_sb, rhs=mask_sb,
                            start=False, stop=True,
                            skip_group_check=True,
                        )
                for h in range(2):
                    wh = h1 - h0
                    nc.scalar.activation(
                        out=e_t[h][:, h0 - s0:h0 - s0 + wh],
                        in_=pss[h][:, 0:wh], func=AF.Exp,
                        scale=SCALE, accum_out=den[:, h, hf:hf + 1],
                    )
            return i, len(halves), e_t, den

        def emit_norm_av(p, st):
            i, nhalves, e_t, den = st
            if 0 not in P[p]["po_half"]:
                open_half(p, 0)
            rec = small.tile([128, 2], fp32, tag="rec", name="rec")
            if nhalves == 2:
                nc.vector.tensor_reduce(
                    out=rec, in_=den[:, :, :],
                    axis=mybir.AxisListType.X, op=ALU.add)
                nc.vector.reciprocal(rec, rec)
            else:
                nc.vector.reciprocal(rec, den[:, :, 0])
            vp = vpool.tile([128, 128], fp16, tag="vp", name="vp")
            for h in range(2):
                nc.vector.tensor_scalar_mul(
                    out=vp[:, 64 * h:64 * (h + 1)],
                    in0=v_sb[:, i, 128 * p + 64 * h:128 * p + 64 * (h + 1)],
                    scalar1=rec[:, h:h + 1],
                )
            emit_av(p, i, e_t, vp, (0, 1) if i <= 7 else (2, 3))
            if i <= 7:
                P[p]["saved"][i] = (e_t, vp)
            if i == 3:
                flush_chunk(p, 0)
            if i == 7:
                flush_chunk(p, 1)
                open_half(p, 1)
            P[p]["navs"] += 1

        def emit_av(p, i, e_t, vp, jlist):
            s0 = 128 * i
            for h in range(2):
                for j, c0, cw in _av_chunks(s0):
                    if j not in jlist:
                        continue
                    ph = 0 if j < 2 else 1
                    base = 1024 * ph
                    nc.tensor.matmul(
                        P[p]["po_half"][ph][64 * h:64 * (h + 1),
                                            c0 - base:c0 - base + cw],
                        lhsT=vp[:, 64 * h:64 * (h + 1)],
                        rhs=e_t[h][:, c0 - s0:c0 - s0 + cw],
                        start=(i == 0), stop=False,
                        tile_position=(0, 64 * h),
                        skip_group_check=True,
                    )

        def flush_chunk(p, j, use_act=False):
            ph = 0 if j < 2 else 1
            base = 1024 * ph
            o_sb = P[p]["o_sb"]
            src_ = P[p]["po_half"][ph][:, 512 * j - base:512 * (j + 1) - base]
            dst_ = o_sb[:, 512 * j:512 * (j + 1)]
            if use_act:
                # tail flush on the (idle-by-then) ACT engine so it overlaps
                # the DVE's final recip/scale chain
                nc.scalar.copy(dst_, src_)
            else:
                nc.vector.tensor_copy(dst_, src_)
            nc.sync.dma_start(out=out[p][:, 512 * j:512 * (j + 1)],
                              in_=o_sb[:, 512 * j:512 * (j + 1)])

        def drain_one_defer():
            for pp_ in range(PAIRS):
                st = P[pp_]
                n = 2 if pp_ == 1 else 1
                while n and 1 in st["po_half"] and st["defer"]:
                    k = st["defer"].popleft()
                    e_t, vp = st["saved"][k]
                    emit_av(pp_, k, e_t, vp, (2, 3))
                    n -= 1
                if n < (2 if pp_ == 1 else 1):
                    return

        def try_flush2(p):
            """Chunk 2 only receives AV from blocks 0-11 (s0<1536): flush it
            as soon as those are all emitted and the defers have drained."""
            st = P[p]
            if (not st["f2"] and 1 in st["po_half"] and not st["defer"]
                    and st["navs"] >= 12):
                flush_chunk(p, 2)
                st["f2"] = True

        def finish_pair(p):
            """Flush the high half once all AV for pair p has been emitted.
            Pair 1's finish runs post-loop when ACT is idle."""
            st = P[p]
            if st["navs"] == TB and not st["defer"]:
                if not st["f2"]:
                    flush_chunk(p, 2, use_act=(p == 1))
                    st["f2"] = True
                if not st["f3"]:
                    flush_chunk(p, 3, use_act=(p == 1))
                    st["f3"] = True

        units = [(0, i) for i in range(TB)] + [(1, i) for i in range(TB)]
        qk1_at = {1: ("q", 0), 2: ("k", 0), 3: ("q", 1), 4: ("k", 1),
                  5: ("q", 2), 6: ("k", 2), 7: ("q", 3), 8: ("k", 3)}
        pend = deque()
        for (p, i) in units:
            pend.append((p, emit_st_exp(p, i)))
            if len(pend) > 2:
                pp_, st = pend.popleft()
                emit_norm_av(pp_, st)
                if st[0] <= 7:
                    P[pp_]["defer"].append(st[0])
            if p == 0:
                emit_v_upto(i)
                if i in qk1_at:
                    emit_qk1_half(qk1_at[i])
            drain_one_defer()
            try_flush2(0)
            try_flush2(1)
            finish_pair(0)
        while pend:
            pp_, st = pend.popleft()
            emit_norm_av(pp_, st)
            if st[0] <= 7:
                P[pp_]["defer"].append(st[0])
        for p in range(PAIRS):
            while P[p]["defer"]:
                drain_one_defer()
            finish_pair(p)


def _build():
    """Build + schedule + compile the (SPMD-identical) program once."""
    if "nc" in _CACHE:
        return _CACHE["nc"]
    import concourse.bacc as bacc
    import concourse.mybir as mybir
    import concourse.tile as tile

    fp32 = mybir.dt.float32
    fp16 = mybir.dt.float16
    nc = bacc.Bacc("TRN2", target_bir_lowering=False, debug=False)
    io = {
        "x_t": nc.dram_tensor("x_t", [128, EC * T], fp16, kind="ExternalInput").ap(),
        "wq": nc.dram_tensor("wq", [128, EC * 256], fp16, kind="ExternalInput").ap(),
        "wk": nc.dram_tensor("wk", [128, EC * 256], fp16, kind="ExternalInput").ap(),
        "wv": nc.dram_tensor("wv", [128, EC * 256], fp16, kind="ExternalInput").ap(),
        "bq": nc.dram_tensor("bq", [PAIRS, 128], fp32, kind="ExternalInput").ap(),
        "bk": nc.dram_tensor("bk", [PAIRS, 128], fp32, kind="ExternalInput").ap(),
        "bv": nc.dram_tensor("bv", [256], fp32, kind="ExternalInput").ap(),
        "out": nc.dram_tensor("out", [PAIRS, 128, T], fp16,
                              kind="ExternalOutput").ap(),
    }
    with tile.TileContext(nc) as tc:
        _emit(tc, io)
    nc.compile()
    _CACHE["nc"] = nc
    return nc


def _pack_rows(a):
    """(E, n) -> (128, EC*n) partition-major contiguous fp16 rows."""
    n = a.shape[1]
    return np.ascontiguousarray(
        a.astype(np.float16)
        .reshape(EC, 128, n).transpose(1, 0, 2).reshape(128, EC * n))


def make_in_maps(inputs_x, k_w, k_b, q_w, q_b, v_w, v_b):
    """Host-side sharding: per-core input dict."""
    xs = [_pack_rows(np.ascontiguousarray(inputs_x[b].T))
          for b in range(B)]
    in_maps = []
    for core in range(NCORES):
        b, g = divmod(core, 4)
        hs = range(4 * g, 4 * g + 4)
        pack_w = lambda w: _pack_rows(
            np.concatenate([w[h] for h in hs], axis=1))
        pack_b2 = lambda bb: np.ascontiguousarray(
            bb[4 * g:4 * g + 4].reshape(PAIRS, 128).astype(np.float32, copy=False))
        in_maps.append({
            "x_t": xs[b],
            "wq": pack_w(q_w), "wk": pack_w(k_w), "wv": pack_w(v_w),
            "bq": pack_b2(q_b), "bk": pack_b2(k_b),
            "bv": np.ascontiguousarray(
                v_b[4 * g:4 * g + 4].reshape(256).astype(np.float32, copy=False)),
        })
    return in_maps


def assemble(core_outs):
    """Gather per-core (PAIRS, 128, T) outputs into the full (B, T, H*D)."""
    out_full = np.empty((B, T, H * D), np.float32)
    for core in range(NCORES):
        b, g = divmod(core, 4)
        o = core_outs[core]
        for p in range(PAIRS):
            out_full[b, :, g * 256 + 128 * p:g * 256 + 128 * (p + 1)] = o[p].T
    return out_full


def kernel(**inputs):
    x = np.asarray(inputs["inputs"], np.float32)
    args = [np.asarray(inputs[k], np.float32)
            for k in ("k_w", "k_b", "q_w", "q_b", "v_w", "v_b")]
    from concourse.bass_utils import run_bass_kernel_spmd

    nc = _build()
    in_maps = make_in_maps(x, *args)
    res = run_bass_kernel_spmd(nc, in_maps, core_ids=list(range(NCORES)))
    return assemble([r["out"] for r in res.results])


# revision 13
# speedup vs baseline: 1.1193x; 1.1193x over previous
"""Trainium2 Bass kernel: causal multi-head attention with softmax over the
QUERY axis (faithful to the reference's softmax(dim=-2) quirk).

Problem shapes: B=2, T=2048, E=1024, H=16, D=64.

Sharding: 8 cores = 2 batches x 4 head-groups (4 heads per core, zero
communication).  Host pre-transposes X to (E, T) per batch and repacks it
(and the weights) into the exact SBUF partition-major layout so every input
DMA is a fully contiguous row copy.

v3 changes over v2 (172us):
  - score matmuls use K=64/M=128 per head (lhsT = kt[64h:64h+64, s0:s0+128])
    -> 2 matmuls per 512-col chunk instead of 4 64x64 quadrants: halves the
    PE column-cycles spent on scores (58us -> 29us).
  - Q/K projections run in fp8e4 DoubleRow mode (2 e-chunks contracted per
    pass, 0.5 cycles/row): x ships as an fp8 copy, wq/wk ship as fp8
    pre-scaled by 32 (keeps them in e4m3 normal range); the 1/32 descale is
    folded into the bias-add tensor_scalar (mult+add).  V projection stays
    fp16 from the fp16 x copy (fp8 V breaks the 2e-2 tolerance).
  - input DMA priority: biases+wv on the DVE queue, wq8/wk8 then x16(hi) on
    the sync queue, x8 then x16(lo) on the gpsimd queue.  Scalar engine
    issues no DMAs (it is the exp bottleneck).

v2 schedule retained (single continuous stream, ACT-paced):
  - phase B projects ONLY pair 0's Q_T/K_T so the exp stream starts early;
    V projections and pair-1's Q/K are PE "feeder" jobs between early units.
  - causal mask applied on PE (identity @ mask matmul into the diagonal
    score block).
  - exp on ACT with fused accum_out => denominators come free.
  - O_T per pair accumulated in a rotating 2-bank PSUM half with the
    deferred high-t AV trick.
"""

import numpy as np
from contextlib import ExitStack

B, T, E, H, D = 2, 2048, 1024, 16, 64
NCORES = 8
PAIRS = 2          # head pairs per core (4 heads)
EC = E // 128      # 8 contraction chunks
EC2 = EC // 2      # 4 DoubleRow chunk-pairs
TB = T // 128      # 16 s-blocks
TC = T // 512      # 4 output column chunks
SCALE = float(D) ** -0.5
W_SCALE = 32.0     # host-side premultiplier on fp8 q/k weights
INV_W = 1.0 / W_SCALE

_CACHE = {}


def _st_chunks(h0, h1):
    """Split [h0, h1) into <=512 pieces aligned to the psum tile's own 512
    grid (tile column 0 is at absolute t=h0)."""
    out = []
    c0 = h0
    while c0 < h1:
        w = min(512, h1 - c0)
        out.append((c0, w))
        c0 += w
    return out


def _av_chunks(s0):
    """Split [s0, 2048) on the absolute 512 grid (psum_o bank alignment)."""
    out = []
    for j in range(s0 // 512, TC):
        c0 = max(s0, 512 * j)
        out.append((j, c0, 512 * (j + 1) - c0))
    return out


def _emit(tc, io):
    """Emit the kernel program into TileContext tc.  io: dict name -> AP."""
    import concourse.bass as bass
    import concourse.mybir as mybir

    nc = tc.nc
    fp32 = mybir.dt.float32
    fp16 = mybir.dt.float16
    fp8 = mybir.dt.float8e4
    AF = mybir.ActivationFunctionType
    ALU = mybir.AluOpType
    DR = mybir.MatmulPerfMode.DoubleRow

    x_t, x_t8, wq, wk, wv = io["x_t"], io["x_t8"], io["wq"], io["wk"], io["wv"]
    bq, bk, bv, out = io["bq"], io["bk"], io["bv"], io["out"]

    with ExitStack() as ctx:
        const = ctx.enter_context(tc.tile_pool(name="const", bufs=1))
        big = ctx.enter_context(tc.tile_pool(name="big", bufs=1))
        epool = ctx.enter_context(tc.tile_pool(name="epool", bufs=11))
        opool = ctx.enter_context(tc.tile_pool(name="opool", bufs=2))
        small = ctx.enter_context(tc.tile_pool(name="small", bufs=24))
        vpool = ctx.enter_context(tc.tile_pool(name="vpool", bufs=18))
        # "ps" slots are (128,1024) = 2 banks each, bufs=3 -> 6 banks.
        pp = ctx.enter_context(tc.tile_pool(name="pp", bufs=3, space="PSUM"))
        # O_T half accumulator: (128,1024) = 2 banks, single slot.
        po = ctx.enter_context(tc.tile_pool(name="po", bufs=1, space="PSUM"))

        # ---------- constants + inputs ----------
        # fp16 X_T feeds the V projection; fp8 X_T feeds the DoubleRow Q/K
        # projections.  Both packed host-side as (128, EC*T) partition-major
        # rows so every DMA is one contiguous copy.
        xt_sb = big.tile([128, EC, T], fp16, tag="xt")
        xt_view = x_t.rearrange("p (c t) -> p c t", c=EC)
        xt8_sb = big.tile([128, EC, T], fp8, tag="xt8")
        xt8_view = x_t8.rearrange("p (c t) -> p c t", c=EC)

        wq_sb = const.tile([128, EC, 256], fp8, tag="wq")
        wk_sb = const.tile([128, EC, 256], fp8, tag="wk")
        wv_sb = const.tile([128, EC, 256], fp16, tag="wv")
        wq_view = wq.rearrange("p (c n) -> p c n", c=EC)
        wk_view = wk.rearrange("p (c n) -> p c n", c=EC)

        # DMA priority over THREE queues (gpsimd / sync / scalar): the fp8 x
        # + fp8 weights gate the pair-0 projections (critical path); fp16 x
        # gates the V feeders (~unit 2); wv + biases are tiny.  Scalar's
        # queue is idle until the first exp (~25us) so it carries the small
        # stuff plus a share of x16.
        nc.scalar.dma_start(out=wq_sb, in_=wq_view)
        nc.scalar.dma_start(out=wk_sb, in_=wk_view)
        bq_sb = const.tile([128, PAIRS], fp32, tag="bq")
        bk_sb = const.tile([128, PAIRS], fp32, tag="bk")
        for p in range(PAIRS):
            nc.scalar.dma_start(out=bq_sb[:, p:p + 1], in_=bq[p, :, None])
            nc.scalar.dma_start(out=bk_sb[:, p:p + 1], in_=bk[p, :, None])
        bv_sb = const.tile([128, 256], fp32, tag="bv")
        bv_bcast = bass.AP(tensor=bv.tensor, offset=bv.offset,
                           ap=[[0, 128]] + list(bv.ap))
        nc.scalar.dma_start(out=bv_sb, in_=bv_bcast)
        nc.scalar.dma_start(out=wv_sb,
                            in_=wv.rearrange("p (c n) -> p c n", c=EC))

        # fp8 x chunk-pairs alternate gpsimd/sync so phase B's c4-outer loop
        # consumes them in arrival order
        for c4 in range(EC2):
            q = nc.gpsimd if c4 % 2 == 0 else nc.sync
            q.dma_start(out=xt8_sb[:, 2 * c4:2 * c4 + 2, :],
                        in_=xt8_view[:, 2 * c4:2 * c4 + 2, :])
        # x16 pieces spread 6/6/4 over gpsimd/sync/scalar
        xq = [nc.gpsimd, nc.sync, nc.gpsimd, nc.sync,
              nc.scalar, nc.gpsimd, nc.sync, nc.scalar]
        for c in range(EC):
            for hh in range(2):
                idx = 2 * c + hh
                xq[idx % 8].dma_start(
                    out=xt_sb[:, c, 1024 * hh:1024 * (hh + 1)],
                    in_=xt_view[:, c, 1024 * hh:1024 * (hh + 1)])

        # PE warm-up: dummy matmuls during the input-DMA wait so the HAM
        # clock gate reaches 2.4 GHz before real work starts
        warm_sb = const.tile([128, 512], fp16, tag="warm")
        nc.vector.memset(warm_sb, 0.0)
        wps = po.tile([128, 1024], fp32, tag="po", name="ps_warm")
        for _ in range(8):
            nc.tensor.matmul(wps[:, 0:512], lhsT=warm_sb[:, 0:128],
                             rhs=warm_sb, start=True, stop=True)

        # mask[s, t] = 0 if t >= s else MNEG, applied on PE as identity@mask
        # (a DVE mask-add head-of-line blocks the ACT queue behind feeder
        # bias-adds; the PE path keeps the exp stream self-contained)
        MNEG = -60000.0
        mask_sb = const.tile([128, 128], fp16, tag="mask")
        nc.vector.memset(mask_sb, 0.0)
        nc.gpsimd.affine_select(
            out=mask_sb, in_=mask_sb,
            pattern=[[1, 128]], channel_multiplier=-1, base=0,
            compare_op=ALU.is_ge, fill=MNEG,
        )
        ident_sb = const.tile([128, 128], fp16, tag="ident")
        nc.vector.memset(ident_sb, 0.0)
        nc.gpsimd.affine_select(
            out=ident_sb, in_=ident_sb,
            pattern=[[1, 128]], channel_multiplier=-1, base=0,
            compare_op=ALU.not_equal, fill=1.0,
        )

        # ---------- phase B: pair-0 Q/K projections (fp8 DoubleRow) -------
        qt_sb = [big.tile([128, T], fp16, tag=f"qt{p}", name=f"qt{p}") for p in range(PAIRS)]
        kt_sb = [big.tile([128, T], fp16, tag=f"kt{p}", name=f"kt{p}") for p in range(PAIRS)]
        v_sb = big.tile([128, TB, 256], fp16, tag="v")

        def proj_bias(dst, src, b_sb, p):
            # dst = src * (1/32) + b   (descale the fp8 weight premultiplier)
            nc.vector.tensor_scalar(
                out=dst, in0=src, scalar1=INV_W, scalar2=b_sb[:, p:p + 1],
                op0=ALU.mult, op1=ALU.add)

        # c4-outermost over all 8 PSUM banks: the projection finishes just
        # after the last x8 chunk-pair lands.
        scq = [pp.tile([128, 1024], fp32, tag="ps", name="scq0"),
               pp.tile([128, 1024], fp32, tag="ps", name="scq1")]
        sck = [pp.tile([128, 1024], fp32, tag="ps", name="sck0"),
               po.tile([128, 1024], fp32, tag="po", name="sck1")]
        groups = ((scq, qt_sb[0], bq_sb, wq_sb), (sck, kt_sb[0], bk_sb, wk_sb))
        for c4 in range(EC2):
            last = c4 == EC2 - 1
            for jh in range(2):
                for scs, dst, b_sb, w_sb in groups:
                    for j2 in range(2):
                        j = 2 * jh + j2
                        for u in range(2):
                            # one psum group per 2KB bank: start on the
                            # first 256-col write, stop on the last
                            nc.tensor.matmul(
                                scs[jh][:, 512 * j2 + 256 * u:
                                        512 * j2 + 256 * u + 256],
                                lhsT=w_sb[:, 2 * c4:2 * c4 + 2, 0:128],
                                rhs=xt8_sb[:, 2 * c4:2 * c4 + 2,
                                           512 * j + 256 * u:
                                           512 * j + 256 * u + 256],
                                start=(c4 == 0 and u == 0),
                                stop=(last and u == 1),
                                perf_mode=DR,
                            )
                        if last:
                            proj_bias(dst[:, 512 * j:512 * (j + 1)],
                                      scs[jh][:, 512 * j2:512 * (j2 + 1)],
                                      b_sb, 0)

        # ---------- feeder jobs (run inside the attention stream) ----------
        vstate = {"next": 0}

        def emit_v_blocks(tbs):
            for tb in tbs:
                ps = pp.tile([128, 1024], fp32, tag="ps", name="ps_v")
                for c in range(EC):
                    nc.tensor.matmul(
                        ps[:, 0:256],
                        lhsT=xt_sb[:, c, 128 * tb:128 * (tb + 1)],
                        rhs=wv_sb[:, c, :],
                        start=(c == 0), stop=(c == EC - 1),
                    )
                nc.vector.tensor_tensor(out=v_sb[:, tb, :], in0=ps[:, 0:256],
                                        in1=bv_sb, op=ALU.add)

        def emit_v_upto(tb_needed):
            while vstate["next"] <= min(tb_needed, TB - 1):
                emit_v_blocks([vstate["next"]])
                vstate["next"] += 1

        def emit_qk1_job(jc):
            """Project pair 1's Q and K for 512-col t-chunk jc (DoubleRow)."""
            sc = pp.tile([128, 1024], fp32, tag="ps", name=f"qk1_{jc}")
            for c4 in range(EC2):
                for u in range(2):
                    col = 512 * jc + 256 * u
                    nc.tensor.matmul(
                        sc[:, 256 * u:256 * u + 256],
                        lhsT=wq_sb[:, 2 * c4:2 * c4 + 2, 128:256],
                        rhs=xt8_sb[:, 2 * c4:2 * c4 + 2, col:col + 256],
                        start=(c4 == 0 and u == 0),
                        stop=(c4 == EC2 - 1 and u == 1),
                        perf_mode=DR,
                    )
                    nc.tensor.matmul(
                        sc[:, 512 + 256 * u:512 + 256 * u + 256],
                        lhsT=wk_sb[:, 2 * c4:2 * c4 + 2, 128:256],
                        rhs=xt8_sb[:, 2 * c4:2 * c4 + 2, col:col + 256],
                        start=(c4 == 0 and u == 0),
                        stop=(c4 == EC2 - 1 and u == 1),
                        perf_mode=DR,
                    )
            proj_bias(qt_sb[1][:, 512 * jc:512 * (jc + 1)], sc[:, 0:512],
                      bq_sb, 1)
            proj_bias(kt_sb[1][:, 512 * jc:512 * (jc + 1)], sc[:, 512:1024],
                      bk_sb, 1)

        # ---------- attention: one merged stream over 32 (pair, block) ----
        P = {}
        for p in range(PAIRS):
            P[p] = {"po_half": {}, "saved": {}, "o_sb": None,
                    "defer": None, "navs": 0, "f2": False, "f3": False}
        from collections import deque
        for p in range(PAIRS):
            P[p]["defer"] = deque()
            P[p]["o_sb"] = opool.tile([128, T], fp16, tag=f"o{p}",
                                      name=f"o_sb{p}")

        def open_half(p, ph):
            t = po.tile([128, 1024], fp32, tag="po", name=f"po_{p}_{ph}")
            P[p]["po_half"][ph] = t

        def emit_st_exp(p, i):
            s0 = 128 * i
            halves = [(h0, min(T, h0 + 1024))
                      for h0 in (s0, s0 + 1024) if h0 < T]
            e_t = [epool.tile([128, T], fp16, tag=f"e{h}", name=f"e{h}")
                   for h in range(2)]
            den = small.tile([128, 2, 2], fp32, tag="den", name="den")
            for hf, (h0, h1) in enumerate(halves):
                pss = [pp.tile([128, 1024], fp32, tag="ps", name="ps_st")
                       for _ in range(2)]
                # one K=64/M=128 matmul per head; the two heads sit in
                # opposite halves of the PE array (tile_position rows).
                # h-outer / chunk-inner keeps lhsT constant across chunks.
                for h in range(2):
                    for c0, cw in _st_chunks(h0, h1):
                        nc.tensor.matmul(
                            pss[h][:, c0 - h0:c0 - h0 + cw],
                            lhsT=kt_sb[p][64 * h:64 * (h + 1), s0:s0 + 128],
                            rhs=qt_sb[p][64 * h:64 * (h + 1), c0:c0 + cw],
                            start=True, stop=True,
                            tile_position=(64 * h, 0),
                            skip_group_check=True,
                        )
                    if hf == 0:
                        # causal mask add on PE: += identity.T @ mask
                        nc.tensor.matmul(
                            pss[h][:, 0:128],
                            lhsT=ident_sb, rhs=mask_sb,
                            start=False, stop=True,
                            skip_group_check=True,
                        )
                for h in range(2):
                    wh = h1 - h0
                    nc.scalar.activation(
                        out=e_t[h][:, h0 - s0:h0 - s0 + wh],
                        in_=pss[h][:, 0:wh], func=AF.Exp,
                        scale=SCALE, accum_out=den[:, h, hf:hf + 1],
                    )
            return i, len(halves), e_t, den

        def emit_norm_av(p, st):
            i, nhalves, e_t, den = st
            if 0 not in P[p]["po_half"]:
                open_half(p, 0)
            rec = small.tile([128, 2], fp32, tag="rec", name="rec")
            if nhalves == 2:
                nc.vector.tensor_reduce(
                    out=rec, in_=den[:, :, :],
                    axis=mybir.AxisListType.X, op=ALU.add)
                nc.vector.reciprocal(rec, rec)
            else:
                nc.vector.reciprocal(rec, den[:, :, 0])
            vp = vpool.tile([128, 128], fp16, tag="vp", name="vp")
            for h in range(2):
                nc.vector.tensor_scalar_mul(
                    out=vp[:, 64 * h:64 * (h + 1)],
                    in0=v_sb[:, i, 128 * p + 64 * h:128 * p + 64 * (h + 1)],
                    scalar1=rec[:, h:h + 1],
                )
            emit_av(p, i, e_t, vp, (0, 1) if i <= 7 else (2, 3))
            if i <= 7:
                P[p]["saved"][i] = (e_t, vp)
            if i == 3:
                flush_chunk(p, 0)
            if i == 7:
                flush_chunk(p, 1)
                open_half(p, 1)
            P[p]["navs"] += 1

        def emit_av(p, i, e_t, vp, jlist):
            s0 = 128 * i
            for h in range(2):
                for j, c0, cw in _av_chunks(s0):
                    if j not in jlist:
                        continue
                    ph = 0 if j < 2 else 1
                    base = 1024 * ph
                    nc.tensor.matmul(
                        P[p]["po_half"][ph][64 * h:64 * (h + 1),
                                            c0 - base:c0 - base + cw],
                        lhsT=vp[:, 64 * h:64 * (h + 1)],
                        rhs=e_t[h][:, c0 - s0:c0 - s0 + cw],
                        start=(i == 0), stop=False,
                        tile_position=(0, 64 * h),
                        skip_group_check=True,
                    )

        def flush_chunk(p, j, use_act=False):
            ph = 0 if j < 2 else 1
            base = 1024 * ph
            o_sb = P[p]["o_sb"]
            src_ = P[p]["po_half"][ph][:, 512 * j - base:512 * (j + 1) - base]
            dst_ = o_sb[:, 512 * j:512 * (j + 1)]
            if use_act:
                # tail flush on the (idle-by-then) ACT engine so it overlaps
                # the DVE's final recip/scale chain
                nc.scalar.copy(dst_, src_)
            else:
                nc.vector.tensor_copy(dst_, src_)
            nc.sync.dma_start(out=out[p][:, 512 * j:512 * (j + 1)],
                              in_=o_sb[:, 512 * j:512 * (j + 1)])

        def drain_one_defer():
            for pp_ in range(PAIRS):
                st = P[pp_]
                n = 2 if pp_ == 1 else 1
                while n and 1 in st["po_half"] and st["defer"]:
                    k = st["defer"].popleft()
                    e_t, vp = st["saved"][k]
                    emit_av(pp_, k, e_t, vp, (2, 3))
                    n -= 1
                if n < (2 if pp_ == 1 else 1):
                    return

        def try_flush2(p):
            """Chunk 2 only receives AV from blocks 0-11 (s0<1536): flush it
            as soon as those are all emitted and the defers have drained."""
            st = P[p]
            if (not st["f2"] and 1 in st["po_half"] and not st["defer"]
                    and st["navs"] >= 12):
                flush_chunk(p, 2)
                st["f2"] = True

        def finish_pair(p):
            """Flush the high half once all AV for pair p has been emitted.
            Pair 1's finish runs post-loop when ACT is idle."""
            st = P[p]
            if st["navs"] == TB and not st["defer"]:
                if not st["f2"]:
                    flush_chunk(p, 2, use_act=(p == 1))
                    st["f2"] = True
                if not st["f3"]:
                    flush_chunk(p, 3, use_act=(p == 1))
                    st["f3"] = True

        units = [(0, i) for i in range(TB)] + [(1, i) for i in range(TB)]
        qk1_at = {1: 0, 3: 1, 5: 2, 7: 3}
        pend = deque()
        for (p, i) in units:
            pend.append((p, emit_st_exp(p, i)))
            if len(pend) > 2:
                pp_, st = pend.popleft()
                emit_norm_av(pp_, st)
                if st[0] <= 7:
                    P[pp_]["defer"].append(st[0])
            if p == 0:
                emit_v_upto(i)
                if i in qk1_at:
                    emit_qk1_job(qk1_at[i])
            drain_one_defer()
            try_flush2(0)
            try_flush2(1)
            finish_pair(0)
        while pend:
            pp_, st = pend.popleft()
            emit_norm_av(pp_, st)
            if st[0] <= 7:
                P[pp_]["defer"].append(st[0])
        for p in range(PAIRS):
            while P[p]["defer"]:
                drain_one_defer()
            finish_pair(p)


def _build():
    """Build + schedule + compile the (SPMD-identical) program once."""
    if "nc" in _CACHE:
        return _CACHE["nc"]
    import concourse.bacc as bacc
    import concourse.mybir as mybir
    import concourse.tile as tile

    fp32 = mybir.dt.float32
    fp16 = mybir.dt.float16
    fp8 = mybir.dt.float8e4
    nc = bacc.Bacc("TRN2", target_bir_lowering=False, debug=False)
    io = {
        "x_t": nc.dram_tensor("x_t", [128, EC * T], fp16, kind="ExternalInput").ap(),
        "x_t8": nc.dram_tensor("x_t8", [128, EC * T], fp8, kind="ExternalInput").ap(),
        "wq": nc.dram_tensor("wq", [128, EC * 256], fp8, kind="ExternalInput").ap(),
        "wk": nc.dram_tensor("wk", [128, EC * 256], fp8, kind="ExternalInput").ap(),
        "wv": nc.dram_tensor("wv", [128, EC * 256], fp16, kind="ExternalInput").ap(),
        "bq": nc.dram_tensor("bq", [PAIRS, 128], fp32, kind="ExternalInput").ap(),
        "bk": nc.dram_tensor("bk", [PAIRS, 128], fp32, kind="ExternalInput").ap(),
        "bv": nc.dram_tensor("bv", [256], fp32, kind="ExternalInput").ap(),
        "out": nc.dram_tensor("out", [PAIRS, 128, T], fp16,
                              kind="ExternalOutput").ap(),
    }
    with tile.TileContext(nc) as tc:
        _emit(tc, io)
    nc.compile()
    _CACHE["nc"] = nc
    return nc


def _pack_rows(a, dtype=np.float16):
    """(E, n) -> (128, EC*n) partition-major contiguous rows."""
    n = a.shape[1]
    return np.ascontiguousarray(
        a.astype(dtype)
        .reshape(EC, 128, n).transpose(1, 0, 2).reshape(128, EC * n))


def make_in_maps(inputs_x, k_w, k_b, q_w, q_b, v_w, v_b):
    """Host-side sharding: per-core input dict."""
    import ml_dtypes
    fp8np = ml_dtypes.float8_e4m3
    xts = [np.ascontiguousarray(np.asarray(inputs_x[b], np.float32).T)
           for b in range(B)]
    xs = [_pack_rows(x) for x in xts]
    xs8 = [_pack_rows(x, fp8np) for x in xts]
    in_maps = []
    for core in range(NCORES):
        b, g = divmod(core, 4)
        hs = range(4 * g, 4 * g + 4)
        pack_w = lambda w, dt_, s: _pack_rows(
            np.concatenate([w[h] * s for h in hs], axis=1), dt_)
        pack_b2 = lambda bb: np.ascontiguousarray(
            bb[4 * g:4 * g + 4].reshape(PAIRS, 128).astype(np.float32, copy=False))
        in_maps.append({
            "x_t": xs[b], "x_t8": xs8[b],
            "wq": pack_w(q_w, fp8np, W_SCALE),
            "wk": pack_w(k_w, fp8np, W_SCALE),
            "wv": pack_w(v_w, np.float16, 1.0),
            "bq": pack_b2(q_b), "bk": pack_b2(k_b),
            "bv": np.ascontiguousarray(
                v_b[4 * g:4 * g + 4].reshape(256).astype(np.float32, copy=False)),
        })
    return in_maps


def assemble(core_outs):
    """Gather per-core (PAIRS, 128, T) outputs into the full (B, T, H*D)."""
    out_full = np.empty((B, T, H * D), np.float32)
    for core in range(NCORES):
        b, g = divmod(core, 4)
        o = core_outs[core]
        for p in range(PAIRS):
            out_full[b, :, g * 256 + 128 * p:g * 256 + 128 * (p + 1)] = o[p].T
    return out_full


def kernel(**inputs):
    x = np.asarray(inputs["inputs"], np.float32)
    args = [np.asarray(inputs[k], np.float32)
            for k in ("k_w", "k_b", "q_w", "q_b", "v_w", "v_b")]
    from concourse.bass_utils import run_bass_kernel_spmd

    nc = _build()
    in_maps = make_in_maps(x, *args)
    res = run_bass_kernel_spmd(nc, in_maps, core_ids=list(range(NCORES)))
    return assemble([r["out"] for r in res.results])
